# revision 10
# baseline (speedup 1.0000x reference)
"""Trainium2 Bass kernel for nn_BatchLinear (segmented path-indexed grouped linear, MoE-routed).

Math (per token b with expert e = w_id[b], 8 paths (i, j, k, alpha)):
    out[b, 128*k:+128] += alpha * x[b, 128*i:+128] @ W[e, seg j]  (each seg 128x128)

Strategy (v5, expert-parallel + int8 activations):
  - Host: route tokens by expert; each expert's tokens are split across 2 of
    the 8 cores (4 experts x 2 cores).  x is quantized to int8 with a
    per-token scale s_t = absmax_t/127; the device computes y_q = x_q @ W
    unscaled (int8 values are exact in bf16) and the host multiplies rows by
    s_t on unpack — so x DMA is 1 byte/elem with no device-side scale work.
    Data is packed chunk-major ([part, chunk, seg, tok]) so every DMA run is
    4-8 KB contiguous per partition.  Path coefficient 0.5 is folded into
    weight segs 4-7 on the host.
  - Device (identical program on all cores, only data differs): one expert
    per core.  Weights (256 KB) + int8 x chunks stream in on the sync HWDGE
    ring; vector+scalar each cast half of a chunk int8->bf16; per 512-token
    tile 8 bf16 matmuls accumulate the 4 output segments in fp32 PSUM
    (2 paths each); vector/scalar alternate PSUM->SBUF bf16 drains; y chunks
    stream out on the scalar HWDGE ring so stores never block loads.
  - Host: scatter rows back to original token order, applying s_t.
"""

import os

import ml_dtypes
import numpy as np

import concourse.bacc as bacc
import concourse.mybir as mybir
import concourse.tile as tile
from concourse.bass_utils import run_bass_kernel_spmd

N_CORES = 8
B = 32768
E = 4
U = V = 128
IN_STRIDE = 512
N_SEG = 4  # input/output feature segments
CORES_PER_EXPERT = N_CORES // E
# out seg k <- (input seg, weight seg) x 2 contributions (coefficients folded
# into the prescaled weights: segs 4-7 are scaled by 0.5 on the host)
CONTRIB = {0: [(0, 0), (3, 7)], 1: [(1, 1), (0, 4)], 2: [(2, 2), (1, 5)], 3: [(3, 3), (2, 6)]}

F32 = mybir.dt.float32
BF16 = mybir.dt.bfloat16
I8 = mybir.dt.int8

_cache = {}


def _chunks(cap):
    """Token chunks: a small 512 lead-in (early compute start), 1024s, and a
    >=528 16-aligned remainder merged into the last chunk (max 1536)."""
    assert cap % 16 == 0 and cap >= 16
    sizes = []
    rest = cap
    if cap >= 1536:
        sizes.append(256)
        rest = cap - 256
    while rest > 1536:
        sizes.append(1024)
        rest -= 1024
    if rest:
        sizes.append(rest)
    out = []
    c0 = 0
    for s in sizes:
        out.append((c0, s))
        c0 += s
    return out


def _build(cap):
    """Build + schedule the per-core Bass program for per-core capacity `cap`
    (one expert per core)."""
    if cap in _cache:
        return _cache[cap]

    nc = bacc.Bacc("TRN2", target_bir_lowering=False, debug=False, num_devices=N_CORES)
    # chunk-major: x[p, 4*c0 : 4*(c0+CH)] is one contiguous [seg, tok] block
    x = nc.dram_tensor("x", [128, N_SEG * cap], I8, kind="ExternalInput")
    # weights pre-packed on the host into the SBUF layout [u, j, v]
    w = nc.dram_tensor("w", [U, 8 * V], BF16, kind="ExternalInput")
    y = nc.dram_tensor("y", [128, N_SEG * cap], BF16, kind="ExternalOutput")

    chunks = _chunks(cap)

    with tile.TileContext(nc) as tc:
        with (
            tc.tile_pool(name="wpool", bufs=1) as wp,
            tc.tile_pool(name="xin", bufs=1) as xp,
            tc.tile_pool(name="xb", bufs=3) as xbp,
            tc.tile_pool(name="yout", bufs=1) as yp,
            tc.tile_pool(name="ps", bufs=2, space="PSUM") as pp,
        ):
            xts = []
            for ci, (c0, CH) in enumerate(chunks):
                xt = xp.tile([128, N_SEG, CH], I8, tag=f"x{ci}")
                nc.sync.dma_start(
                    xt[:],
                    x[:, N_SEG * c0 : N_SEG * (c0 + CH)].rearrange(
                        "p (s t) -> p s t", t=CH
                    ),
                )
                xts.append(xt)
                if ci == 0:
                    wt = wp.tile([U, 8, V], BF16, tag="w", name="wt")
                    nc.sync.dma_start(wt[:], w.rearrange("u (j v) -> u j v", v=V))

            # PE warm-up during the initial DMA wait: dummy matmuls flip the
            # HAM clock gate to 8/8 before the first real matmul arrives
            dwu = wp.tile([U, V], BF16, name="dwu")
            dxu = wp.tile([128, 512], BF16, name="dxu")
            nc.gpsimd.memset(dwu[:], 0.0)
            nc.gpsimd.memset(dxu[:], 0.0)
            ps_warm = pp.tile([128, 2, 512], F32, tag="ps", name="ps_warm")
            for _ in range(10):
                nc.tensor.matmul(ps_warm[:, 0, :], dwu[:], dxu[:], start=True, stop=True)

            for ci, (c0, CH) in enumerate(chunks):
                xt = xts[ci]
                # cast int8 -> bf16 (exact for |v| <= 127) on the DVE
                # (2 elem/cycle; ACT only manages 1)
                xb = xbp.tile([128, N_SEG, 1536], BF16, tag="xb")
                nc.vector.tensor_copy(xb[:, :, :CH], xt[:])
                ys = yp.tile([128, N_SEG, CH], BF16, tag=f"y{ci}")
                for t0 in range(0, CH, 512):
                    T = min(512, CH - t0)
                    # two half-tiles of PSUM (2 banks each, 4 bufs total) so
                    # the k01 drain overlaps the k23 matmuls
                    for h in range(2):
                        ps = pp.tile([128, 2, 512], F32, tag="ps")
                        for kh in range(2):
                            k = 2 * h + kh
                            (i1, j1), (i2, j2) = CONTRIB[k]
                            nc.tensor.matmul(
                                ps[:, kh, :T],
                                wt[:, j1, :],
                                xb[:, i1, t0 : t0 + T],
                                start=True,
                                stop=False,
                            )
                            nc.tensor.matmul(
                                ps[:, kh, :T],
                                wt[:, j2, :],
                                xb[:, i2, t0 : t0 + T],
                                start=False,
                                stop=True,
                            )
                        # drains ~70/30 ACT:DVE (DVE also carries the casts)
                        if h == 0:
                            nc.scalar.copy(
                                ys[:, 0:2, t0 : t0 + T], ps[:, :, :T]
                            )
                        else:
                            nc.scalar.copy(ys[:, 2, t0 : t0 + T], ps[:, 0, :T])
                            nc.vector.tensor_copy(
                                ys[:, 3, t0 : t0 + T], ps[:, 1, :T]
                            )
                # stores dispatch from the (otherwise idle) sync engine: a
                # store whose drain isn't ready must not head-of-line block
                # the scalar engine's cast/drain work
                nc.sync.dma_start(
                    y[:, N_SEG * c0 : N_SEG * (c0 + CH)].rearrange(
                        "p (s t) -> p s t", t=CH
                    ),
                    ys[:],
                )

    nc.compile()
    _cache[cap] = nc
    return nc


def _route(tensor_w_id):
    """Expert-parallel routing: expert e's tokens split across cores
    2e and 2e+1.  Returns (chunks, cap): chunks[c] = token indices for
    core c (expert c // 2)."""
    chunks = [None] * N_CORES
    max_n = 1
    for e in range(E):
        idx_e = np.flatnonzero(tensor_w_id == e)
        parts = np.array_split(idx_e, CORES_PER_EXPERT)
        for h in range(CORES_PER_EXPERT):
            c = e * CORES_PER_EXPERT + h
            chunks[c] = parts[h]
            max_n = max(max_n, len(parts[h]))
    cap = -(-max_n // 16) * 16
    return chunks, cap


def _run(tensor_in, tensor_w, tensor_w_id, trace=False):
    tensor_in = np.ascontiguousarray(tensor_in, dtype=np.float32)
    tensor_w = np.asarray(tensor_w, dtype=np.float32)
    tensor_w_id = np.asarray(tensor_w_id, dtype=np.int32)

    chunks, cap = _route(tensor_w_id)
    nc = _build(cap)
    chunk_list = _chunks(cap)

    # prescale: fold the 0.5 path coefficient into weight segs 4-7, and
    # pre-arrange into the SBUF layout [u, j, v] per expert
    w_pack = tensor_w.reshape(E, 8, U, V).copy()
    w_pack[:, 4:] *= 0.5
    w_pack = np.ascontiguousarray(w_pack.transpose(0, 2, 1, 3))  # [e, u, j, v]
    w_pack = w_pack.reshape(E, U, 8 * V).astype(ml_dtypes.bfloat16)

    # per-token int8 quantization: x ~= x_q * s_t
    scale = np.abs(tensor_in).max(axis=1) / 127.0  # [B]
    np.maximum(scale, 1e-30, out=scale)
    x_q = np.rint(tensor_in / scale[:, None]).astype(np.int8)  # [B, 512]

    # pack: gather + transpose to chunk-major [part, chunk, seg, tok] per core
    big_idx = np.zeros((N_CORES, cap), dtype=np.int64)
    for c in range(N_CORES):
        big_idx[c, : len(chunks[c])] = chunks[c]
    xg = x_q[big_idx.reshape(-1)]  # [N_CORES*cap, 512]
    xg = xg.reshape(N_CORES, cap, N_SEG, U)  # [c, tok, seg, part]
    x_pack = np.empty((N_CORES, 128, N_SEG * cap), dtype=np.int8)
    for c0, CH in chunk_list:
        blk = xg[:, c0 : c0 + CH].transpose(0, 3, 2, 1)  # [c, part, seg, tok]
        x_pack[:, :, N_SEG * c0 : N_SEG * (c0 + CH)] = blk.reshape(
            N_CORES, 128, N_SEG * CH
        )

    in_maps = [{"x": x_pack[c], "w": w_pack[c // CORES_PER_EXPERT]} for c in range(N_CORES)]

    kwargs = {}
    if trace:
        import shutil

        os.environ.pop("BASS_NEVER_TRACE", None)
        tmpdir = "/tmp/prof"
        shutil.rmtree(tmpdir, ignore_errors=True)
        os.makedirs(tmpdir, exist_ok=True)
        kwargs["tmpdir"] = tmpdir
    else:
        # a stray BASS_TRACE in the environment would route through the NTFF
        # profile hook, which this image lacks — force tracing off
        os.environ["BASS_NEVER_TRACE"] = "1"
    res = run_bass_kernel_spmd(nc, in_maps, list(range(N_CORES)), trace=trace, **kwargs)

    out = np.empty((B, IN_STRIDE), dtype=np.float32)
    y_all = np.empty((128, N_SEG, cap), dtype=np.float32)
    for c in range(N_CORES):
        idx = chunks[c]
        if not len(idx):
            continue
        yc = np.asarray(res.results[c]["y"])  # [128, N_SEG*cap] bf16, chunk-major
        for c0, CH in chunk_list:
            y_all[:, :, c0 : c0 + CH] = (
                yc[:, N_SEG * c0 : N_SEG * (c0 + CH)]
                .reshape(128, N_SEG, CH)
                .astype(np.float32)
            )
        # y_all[v, s, t] -> out[token, s*128+v]
        flat = y_all.transpose(1, 0, 2).reshape(IN_STRIDE, cap)  # [feat, tok]
        out[idx] = flat[:, : len(idx)].T * scale[idx][:, None]
    return out, res


def kernel(tensor_in, tensor_w, tensor_w_id):
    out, _ = _run(tensor_in, tensor_w, tensor_w_id)
    return out
